# revision 33
# baseline (speedup 1.0000x reference)
"""BiLSTM (eval-mode, dropout inactive) Trainium2 kernel — 8 NeuronCores.

Problem: x [64, 512, 1024] f32; forward + backward LSTM (H=1024) over
S=512 steps; output [64, 512, 2048] f32.

Sharding: pure data-parallel. Cores 0-3 run the forward LSTM, cores 4-7
the backward LSTM (on time-reversed input); within each direction the
batch (64) is split into 4 quarters of 16. Each core holds its full
per-direction weights and runs the whole recurrence for its batch
quarter — no cross-core communication (measured remote-DMA latency on
this fabric, ~13-16 us/hop, makes per-step tensor-parallel exchange
slower than streaming the full Whh per core).

Per core, one SPMD program, two phases:
  1. pre^T[gate, token] = Wih^T x^T + (bih + bhh), one big GEMM
     (tokens = S*16), staged to DRAM in bf16.
  2. 512 sequential steps: gates^T = Whh^T h^T + pre. The recurrent
     weights and h are fp8 E3M4 (Whh scaled x256, h scaled x8 to stay
     in the normal range; descale 1/2048 fused into the psum+pre add),
     which halves the per-matmul LDWEIGHTS time — the phase-2
     bottleneck at moving-dim 16. PSUM for a whole step is one
     [128, 8, 64] f32 tile (4 banks), so the gate add / sigmoid / tanh
     / cell update run as 2 consolidated chunks of 4 h-blocks instead
     of 8 small per-block chains (DVE/ACT instruction overhead
     dominates at [128,16] granularity). h is kept in bf16 for output
     staging (quantizing the stored output itself to fp8 costs ~3x in
     final accuracy); only the recurrent copy fed to the matmul is fp8.
Gate columns are pre-permuted host-side to [i_q f_q o_q g_q] blocks of
128. h^T [1024, 16] per step is staged to DRAM; the host assembles the
final output.
"""
import sys

sys.path.insert(0, "/opt/trn_rl_repo")

import numpy as np
import ml_dtypes

from concourse import bass, bacc, tile, bass_utils

mybir = bass.mybir
BF16 = mybir.dt.bfloat16
F32 = mybir.dt.float32
F8E3 = mybir.dt.float8e3
F8E4 = mybir.dt.float8e4
AF = mybir.ActivationFunctionType
ALU = mybir.AluOpType

bfloat16 = ml_dtypes.bfloat16

_DT = {"bf16": (BF16, ml_dtypes.bfloat16),
       "f8e3": (F8E3, ml_dtypes.float8_e3m4),
       "f8e4": (F8E4, ml_dtypes.float8_e4m3)}

B = 64
S = 512
E = 1024
H = 1024
NCORES = 8
BL = 16                 # batch rows per core (4 quarters per direction)
MT = 32                 # gate-column tiles of 128 (4H / 128)
KT = 8                  # contraction tiles (E == H == 1024)
NQ = 8                  # h sub-blocks of 128 (H / 128)
NPAR = 2                # h^T double buffer
TS = 512                # phase-1 token-tile size
KB = KT * BL

WHH_DT = "bf16"         # "f8e3" | "f8e4" | "bf16"  (recurrent weight dtype)
HT_DT = "bf16"          # "f8e3" | "f8e4" | "bf16"  (recurrent h dtype)
# fp8 note: both e3m4 and e4m3 recurrent weights were tried on HW — both
# produce numerically corrupted matmul results (rel err ~5e-2) AND run
# LDWEIGHTS *slower* than bf16 (33ns vs 27ns per 128-col tile), so fp8
# is a dead end for this kernel.
W_SCALE = {"bf16": 1.0, "f8e3": 256.0, "f8e4": 2048.0}[WHH_DT]
H_SCALE = {"bf16": 1.0, "f8e3": 8.0, "f8e4": 16.0}[HT_DT]
DESCALE = 1.0 / (W_SCALE * H_SCALE)

NCH = 4                 # elementwise chunks per step
QC = NQ // NCH          # h-blocks per chunk (2)

TRACE = False           # set True (e.g. from test.py) to capture NTFF timing
LAST_EXEC_NS = None

_cache = {}


def _build_program():
    nc = bacc.Bacc("TRN2", target_bir_lowering=False, debug=False,
                   num_devices=NCORES)
    NT = S * BL // TS

    xT_d = nc.dram_tensor("xT", [E, S * BL], BF16, kind="ExternalInput")
    wih_d = nc.dram_tensor("wih", [128, KT * MT * 128], BF16, kind="ExternalInput")
    whh_d = nc.dram_tensor("whh", [128, KT * MT * 128], _DT[WHH_DT][0],
                           kind="ExternalInput")
    bias_d = nc.dram_tensor("bias", [128, MT], F32, kind="ExternalInput")
    stage_d = nc.dram_tensor("stage", [S, 128, NQ, BL], BF16, kind="ExternalOutput")
    pre_d = nc.dram_tensor("pre_stage", [MT, 128, S, BL], BF16, kind="Internal")

    with tile.TileContext(nc) as tc:
        with (
            tc.tile_pool(name="persist", bufs=1) as persist,
            tc.tile_pool(name="pre", bufs=2) as prep,
            tc.tile_pool(name="ew", bufs=3) as ewp,
        ):
            whh_dt = _DT[WHH_DT][0]
            ht_dt = _DT[HT_DT][0]
            wih_sb = persist.tile([128, KT * MT * 128], BF16)
            whh_sb = persist.tile([128, KT * MT * 128], whh_dt)
            bias_sb = persist.tile([128, MT], F32)
            # per-chunk recurrent state tiles: separate tiles so the Tile
            # framework's dependency tracking stays chunk-granular (a single
            # big tile serializes next-step matmuls behind the whole
            # elementwise chain)
            hT = [persist.tile([128, NPAR, QC, BL], ht_dt, name="hT%d" % c)
                  for c in range(NCH)]
            c_sb = [persist.tile([128, 2, QC, BL], F32, name="c%d" % c)
                    for c in range(NCH)]

            nc.sync.dma_start(wih_sb[:], wih_d[:])
            nc.sync.dma_start(whh_sb[:], whh_d[:])
            nc.sync.dma_start(bias_sb[:], bias_d[:])

            # ---------------- Phase 1: input projection ----------------
            with (
                tc.tile_pool(name="xt", bufs=2) as xtp,
                tc.tile_pool(name="p1psum", bufs=8, space="PSUM") as p1psum,
                tc.tile_pool(name="p1ev", bufs=8) as p1ev,
            ):
                SPT = TS // BL
                for n in range(NT):
                    xt = xtp.tile([128, KT, TS], BF16)
                    for k in range(KT):
                        nc.sync.dma_start(
                            xt[:, k, :],
                            xT_d[k * 128:(k + 1) * 128, n * TS:(n + 1) * TS])
                    for m in range(MT):
                        ps = p1psum.tile([128, TS], F32)
                        for k in range(KT):
                            nc.tensor.matmul(
                                ps[:],
                                wih_sb[:, (k * MT + m) * 128:(k * MT + m + 1) * 128],
                                xt[:, k, :],
                                start=(k == 0), stop=(k == KT - 1))
                        ev = p1ev.tile([128, TS], BF16)
                        nc.scalar.activation(ev[:], ps[:], AF.Identity,
                                             bias=bias_sb[:, m:m + 1], scale=1.0)
                        nc.sync.dma_start(
                            pre_d[m, :, n * SPT:(n + 1) * SPT, :], ev[:])

            # ---------------- Phase 2: recurrence ----------------
            with tc.tile_pool(name="p2psum", bufs=2, space="PSUM") as p2psum:
                pb = None
                for t in range(S):
                    par = t % NPAR
                    par1 = (t - 1) % NPAR
                    cpo = (t - 1) % 2
                    cpn = t % 2
                    tt = t % 8
                    if tt == 0:
                        # pre tile for next 8 steps: [128, q, mi, tt, b]
                        pb = prep.tile([128, NQ, 4, 8, BL], BF16)
                        for m in range(MT):
                            nc.sync.dma_start(pb[:, m // 4, m % 4, :, :],
                                              pre_d[m, :, t:t + 8, :])

                    psA = psB = None
                    if t > 0:
                        # PSUM has_written bits are cleared bank-wide by each
                        # accumulation group's first (start=True) matmul, so
                        # groups sharing a bank MUST run their matmuls
                        # strictly sequentially. To still overlap the
                        # recurrence across steps, each group's 8-kap
                        # accumulation is split into two banks: psA takes
                        # kaps 0-3 (needs only the previous step's first two
                        # h chunks, which finish early) and psB takes kaps
                        # 4-7. Groups are sequential within each bank; the
                        # gate add combines A + B + pre. Chunks close
                        # staggered (c0 first) so the elementwise chains
                        # overlap the matmul tail and the next step starts
                        # with zero stall.
                        psA = [p2psum.tile([128, QC, 4, BL], F32, bufs=1,
                                           name="psA%d" % c, tag="psA%d" % c)
                               for c in range(NCH)]
                        psB = [p2psum.tile([128, QC, 4, BL], F32, bufs=1,
                                           name="psB%d" % c, tag="psB%d" % c)
                               for c in range(NCH)]

                        def mm(ps, q, mi, kap, start, stop):
                            m = q * 4 + mi
                            nc.tensor.matmul(
                                ps[:, q % QC, mi, :],
                                whh_sb[:, (kap * MT + m) * 128:
                                       (kap * MT + m + 1) * 128],
                                hT[kap // QC][:, par1, kap % QC, :],
                                start=start, stop=stop)

                        # A block first: depends only on the previous step's
                        # chunks 0-1 h tiles, which complete before the
                        # previous matmul stream ends -> zero-stall step
                        # start. Groups (4 consecutive kaps) are strictly
                        # sequential within each block, so bank-wide
                        # has_written clears never wipe a live group.
                        for c in range(NCH):
                            for q in (QC * c, QC * c + 1):
                                for mi in range(4):
                                    for kap in range(4):
                                        mm(psA[c], q, mi, kap,
                                           kap == 0, kap == 3)
                        for c in range(NCH):
                            for q in (QC * c, QC * c + 1):
                                for mi in range(4):
                                    for kap in range(4, KT):
                                        mm(psB[c], q, mi, kap,
                                           kap == 4, kap == KT - 1)

                    import contextlib
                    for cq in range(NCH):
                        qs = cq * QC          # first h-block of chunk
                        # chunk 0 gates the next step's first matmuls, so
                        # bias the scheduler to complete its chain first
                        prio = (tc.high_priority() if cq == 0
                                else contextlib.nullcontext())
                        with prio:
                            if t > 0:
                                # gates = psA*descale + pre, then + psB*descale
                                # (two ops, one PSUM operand each — PSUM has a
                                # single DVE read port; the first add runs
                                # early, hidden under the matmul stream)
                                gh = ewp.tile([128, QC, 4, BL], F32,
                                              tag="gh%d" % cq)
                                nc.vector.scalar_tensor_tensor(
                                    gh[:], psA[cq][:], DESCALE,
                                    pb[:, qs:qs + QC, :, tt, :],
                                    op0=ALU.mult, op1=ALU.add)
                                g = ewp.tile([128, QC, 4, BL], BF16,
                                             tag="g%d" % cq)
                                nc.vector.scalar_tensor_tensor(
                                    g[:], psB[cq][:], DESCALE, gh[:],
                                    op0=ALU.mult, op1=ALU.add)
                                g_sig = g[:, :, 0:3, :]
                                g_tanh = g[:, :, 3, :]
                            else:
                                g_sig = pb[:, qs:qs + QC, 0:3, tt, :]
                                g_tanh = pb[:, qs:qs + QC, 3, tt, :]
                            sig = ewp.tile([128, QC, 3, BL], BF16,
                                           tag="sig%d" % cq)
                            nc.scalar.activation(sig[:], g_sig, AF.Sigmoid)
                            tg = ewp.tile([128, QC, BL], BF16, tag="tg%d" % cq)
                            nc.scalar.activation(tg[:], g_tanh, AF.Tanh)

                            si = sig[:, :, 0, :]
                            sf = sig[:, :, 1, :]
                            so = sig[:, :, 2, :]
                            c_new = c_sb[cq][:, cpn, :, :]
                            if t > 0:
                                c_old = c_sb[cq][:, cpo, :, :]
                                t1 = ewp.tile([128, QC, BL], F32,
                                              tag="t1%d" % cq)
                                nc.vector.tensor_mul(t1[:], si, tg[:])
                                t2 = ewp.tile([128, QC, BL], F32,
                                              tag="t2%d" % cq)
                                nc.vector.tensor_mul(t2[:], sf, c_old)
                                nc.vector.tensor_add(c_new, t1[:], t2[:])
                            else:
                                nc.vector.tensor_mul(c_new, si, tg[:])
                            tc_ = ewp.tile([128, QC, BL], BF16,
                                           tag="tc%d" % cq)
                            nc.scalar.activation(tc_[:], c_new, AF.Tanh)
                            # h (bf16): recurrent input and output staging
                            nc.vector.tensor_mul(hT[cq][:, par, :, :],
                                                 so, tc_[:])
                        nc.sync.dma_start(stage_d[t, :, qs:qs + QC, :],
                                          hT[cq][:, par, :, :])

    nc.compile()
    return nc


def _host_inputs(x, Wih_f, bih_f, Whh_f, bhh_f, Wih_b, bih_b, Whh_b, bhh_b):
    # gate-column permutation: NQ blocks q of [i_q f_q o_q g_q] x 128
    # (reference gate order along 4H is [i, f, g, o])
    cols = []
    for q in range(NQ):
        for goff in (0, H, 3 * H, 2 * H):   # i, f, o, g
            s0 = goff + q * 128
            cols.extend(range(s0, s0 + 128))
    cols = np.array(cols)

    def tiles(w, dt, scale=1.0):
        return np.ascontiguousarray(
            (w * scale).reshape(KT, 128, MT, 128).transpose(1, 0, 2, 3)
            .reshape(128, KT * MT * 128)).astype(dt)

    per_dir = {}
    for fwd, (Wih, bih, Whh, bhh) in (
            (True, (Wih_f, bih_f, Whh_f, bhh_f)),
            (False, (Wih_b, bih_b, Whh_b, bhh_b))):
        per_dir[fwd] = (
            tiles(Wih[:, cols], bfloat16),
            tiles(Whh[:, cols], _DT[WHH_DT][1], W_SCALE),
            np.ascontiguousarray(
                (bih + bhh)[cols].reshape(MT, 128).T).astype(np.float32),
        )

    in_maps = []
    for c in range(NCORES):
        fwd = c < 4
        qb = c & 3
        xs = x[qb * BL:(qb + 1) * BL]
        if not fwd:
            xs = xs[:, ::-1]
        xT = np.ascontiguousarray(
            xs.transpose(2, 1, 0).reshape(E, S * BL)).astype(bfloat16)
        wih_t, whh_t, bias_t = per_dir[fwd]
        in_maps.append({"xT": xT, "wih": wih_t, "whh": whh_t, "bias": bias_t})
    return in_maps


def _assemble(results):
    out = np.empty((B, S, 2 * H), np.float32)
    for c in range(NCORES):
        fwd = c < 4
        qb = c & 3
        arr = np.asarray(results[c]["stage"]).astype(np.float32)
        part = arr.transpose(3, 0, 2, 1).reshape(BL, S, NQ * 128)
        if not fwd:
            part = part[:, ::-1, :]
        base = 0 if fwd else H
        out[qb * BL:(qb + 1) * BL, :, base:base + H] = part
    return out


def kernel(x, Wih_f, bih_f, Whh_f, bhh_f, Wih_b, bih_b, Whh_b, bhh_b):
    global LAST_EXEC_NS
    if "nc" not in _cache:
        _cache["nc"] = _build_program()
    nc = _cache["nc"]
    in_maps = _host_inputs(np.asarray(x, np.float32),
                           np.asarray(Wih_f, np.float32),
                           np.asarray(bih_f, np.float32),
                           np.asarray(Whh_f, np.float32),
                           np.asarray(bhh_f, np.float32),
                           np.asarray(Wih_b, np.float32),
                           np.asarray(bih_b, np.float32),
                           np.asarray(Whh_b, np.float32),
                           np.asarray(bhh_b, np.float32))
    res = bass_utils.run_bass_kernel_spmd(nc, in_maps,
                                          core_ids=list(range(NCORES)),
                                          trace=TRACE)
    LAST_EXEC_NS = res.exec_time_ns
    return _assemble(res.results)


# revision 35
# speedup vs baseline: 1.0450x; 1.0450x over previous
"""BiLSTM (eval-mode, dropout inactive) Trainium2 kernel — 8 NeuronCores.

Problem: x [64, 512, 1024] f32; forward + backward LSTM (H=1024) over
S=512 steps; output [64, 512, 2048] f32.

Sharding: pure data-parallel. Cores 0-3 run the forward LSTM, cores 4-7
the backward LSTM (on time-reversed input); within each direction the
batch (64) is split into 4 quarters of 16. Each core holds its full
per-direction weights and runs the whole recurrence for its batch
quarter — no cross-core communication (measured remote-DMA latency on
this fabric, ~13-16 us/hop, makes per-step tensor-parallel exchange
slower than streaming the full Whh per core).

Per core, one SPMD program, two phases:
  1. pre^T[gate, token] = Wih^T x^T + (bih + bhh), one big GEMM
     (tokens = S*16), staged to DRAM in bf16.
  2. 512 sequential steps: gates^T = Whh^T h^T (+ pre via DVE add),
     sigmoid/tanh on ScalarE, cell update on VectorE. Weights/h in
     bf16 (fp32 PSUM accumulate), cell state c in fp32.
Gate columns are pre-permuted host-side to [i_q f_q o_q g_q] blocks of
128 so sigmoid/tanh each run on contiguous slices. h^T [1024, 16] per
step is staged to DRAM; the host assembles the final output.
"""
import sys

sys.path.insert(0, "/opt/trn_rl_repo")

import numpy as np
import ml_dtypes

from concourse import bass, bacc, tile, bass_utils

mybir = bass.mybir
BF16 = mybir.dt.bfloat16
F32 = mybir.dt.float32
AF = mybir.ActivationFunctionType

bfloat16 = ml_dtypes.bfloat16

B = 64
S = 512
E = 1024
H = 1024
NCORES = 8
BL = 16                 # batch rows per core (4 quarters per direction)
MT = 32                 # gate-column tiles of 128 (4H / 128)
KT = 8                  # contraction tiles (E == H == 1024)
NQ = 8                  # h sub-blocks of 128 (H / 128)
NPAR = 2                # h^T double buffer
TS = 512                # phase-1 token-tile size
KB = KT * BL

TRACE = False           # set True (e.g. from test.py) to capture NTFF timing
LAST_EXEC_NS = None

_cache = {}


def _build_program():
    nc = bacc.Bacc("TRN2", target_bir_lowering=False, debug=False,
                   num_devices=NCORES)
    NT = S * BL // TS

    xT_d = nc.dram_tensor("xT", [E, S * BL], BF16, kind="ExternalInput")
    wih_d = nc.dram_tensor("wih", [128, KT * MT * 128], BF16, kind="ExternalInput")
    whh_d = nc.dram_tensor("whh", [128, KT * MT * 128], BF16, kind="ExternalInput")
    bias_d = nc.dram_tensor("bias", [128, MT], F32, kind="ExternalInput")
    stage_d = nc.dram_tensor("stage", [S, 128, NQ, BL], BF16, kind="ExternalOutput")
    pre_d = nc.dram_tensor("pre_stage", [MT, 128, S, BL], BF16, kind="Internal")

    with tile.TileContext(nc) as tc:
        with (
            tc.tile_pool(name="persist", bufs=1) as persist,
            tc.tile_pool(name="pre", bufs=2) as prep,
            tc.tile_pool(name="ew", bufs=3) as ewp,
        ):
            wih_sb = persist.tile([128, KT * MT * 128], BF16)
            whh_sb = persist.tile([128, KT * MT * 128], BF16)
            bias_sb = persist.tile([128, MT], F32)
            hT = persist.tile([128, NPAR * KB], BF16)
            c_sb = persist.tile([128, 2 * NQ * BL], F32)

            nc.sync.dma_start(wih_sb[:], wih_d[:])
            nc.sync.dma_start(whh_sb[:], whh_d[:])
            nc.sync.dma_start(bias_sb[:], bias_d[:])

            # Phase 1 (input projection) and phase 2 (recurrence) share one
            # scope: only the first token chunk runs up front; the remaining
            # 15 chunks are emitted at deprioritized scheduler priority so
            # their matmuls fill the tensor-engine idle gaps in the
            # recurrence (the per-step elementwise tail) instead of running
            # serially before it.
            SPT = TS // BL
            STEPS_PER_NT = TS // BL   # 32 time steps covered per token chunk

            with (
                tc.tile_pool(name="xt", bufs=2) as xtp,
                tc.tile_pool(name="p1psum", bufs=2, space="PSUM") as p1psum,
                tc.tile_pool(name="p1ev", bufs=4) as p1ev,
                tc.tile_pool(name="p2psum", bufs=6, space="PSUM") as p2psum,
            ):
                def phase1_chunk(n):
                    xt = xtp.tile([128, KT, TS], BF16, name="xt")
                    for k in range(KT):
                        nc.sync.dma_start(
                            xt[:, k, :],
                            xT_d[k * 128:(k + 1) * 128, n * TS:(n + 1) * TS])
                    for m in range(MT):
                        ps1 = p1psum.tile([128, TS], F32, name="ps1")
                        for k in range(KT):
                            nc.tensor.matmul(
                                ps1[:],
                                wih_sb[:, (k * MT + m) * 128:(k * MT + m + 1) * 128],
                                xt[:, k, :],
                                start=(k == 0), stop=(k == KT - 1))
                        ev = p1ev.tile([128, TS], BF16, name="ev")
                        nc.scalar.activation(ev[:], ps1[:], AF.Identity,
                                             bias=bias_sb[:, m:m + 1], scale=1.0)
                        nc.sync.dma_start(
                            pre_d[m, :, n * SPT:(n + 1) * SPT, :], ev[:])

                phase1_chunk(0)

                pb = None
                for t in range(S):
                    par = t % NPAR
                    par1 = (t - 1) % NPAR
                    cpo = (t - 1) % 2
                    cpn = t % 2
                    tt = t % 8
                    if t % STEPS_PER_NT == 0 and t // STEPS_PER_NT + 1 < NT:
                        # emit the next token chunk as low-priority filler
                        # (scheduler picks it only when no recurrence work
                        # is ready)
                        with tc.high_priority(offset=-1_000_000):
                            phase1_chunk(t // STEPS_PER_NT + 1)
                    if tt == 0:
                        pb = prep.tile([128, MT, 8, BL], BF16)
                        for m in range(MT):
                            nc.sync.dma_start(pb[:, m, :, :],
                                              pre_d[m, :, t:t + 8, :])

                    qps = []
                    if t > 0:
                        for q in range(NQ):
                            ps = p2psum.tile([128, 4 * BL], F32)
                            for mi in range(4):
                                m = q * 4 + mi
                                for kap in range(KT):
                                    nc.tensor.matmul(
                                        ps[:, mi * BL:(mi + 1) * BL],
                                        whh_sb[:, (kap * MT + m) * 128:
                                               (kap * MT + m + 1) * 128],
                                        hT[:, par1 * KB + kap * BL:
                                           par1 * KB + (kap + 1) * BL],
                                        start=(kap == 0), stop=(kap == KT - 1))
                            qps.append(ps)

                    for q in range(NQ):
                        if t > 0:
                            g = ewp.tile([128, 4 * BL], BF16, tag="g")
                            nc.vector.tensor_add(g[:], qps[q][:],
                                                 pb[:, q * 4:q * 4 + 4, tt, :])
                            g_sig = g[:, 0:3 * BL]
                            g_tanh = g[:, 3 * BL:4 * BL]
                        else:
                            g_sig = pb[:, q * 4:q * 4 + 3, tt, :]
                            g_tanh = pb[:, q * 4 + 3, tt, :]
                        sig = ewp.tile([128, 3 * BL], BF16, tag="sig")
                        nc.scalar.activation(sig[:], g_sig, AF.Sigmoid)
                        tg = ewp.tile([128, BL], BF16, tag="tg")
                        nc.scalar.activation(tg[:], g_tanh, AF.Tanh)

                        c_new = c_sb[:, (cpn * NQ + q) * BL:(cpn * NQ + q + 1) * BL]
                        if t > 0:
                            c_old = c_sb[:, (cpo * NQ + q) * BL:
                                         (cpo * NQ + q + 1) * BL]
                            t1 = ewp.tile([128, BL], F32, tag="t1")
                            nc.vector.tensor_mul(t1[:], sig[:, 0:BL], tg[:])
                            t2 = ewp.tile([128, BL], F32, tag="t2")
                            nc.vector.tensor_mul(t2[:], sig[:, BL:2 * BL], c_old)
                            nc.vector.tensor_add(c_new, t1[:], t2[:])
                        else:
                            nc.vector.tensor_mul(c_new, sig[:, 0:BL], tg[:])
                        tc_ = ewp.tile([128, BL], BF16, tag="tc")
                        nc.scalar.activation(tc_[:], c_new, AF.Tanh)
                        nc.vector.tensor_mul(
                            hT[:, par * KB + q * BL:par * KB + (q + 1) * BL],
                            sig[:, 2 * BL:3 * BL], tc_[:])

                    nc.sync.dma_start(stage_d[t],
                                      hT[:, par * KB:par * KB + NQ * BL])

    nc.compile()
    return nc


def _host_inputs(x, Wih_f, bih_f, Whh_f, bhh_f, Wih_b, bih_b, Whh_b, bhh_b):
    # gate-column permutation: NQ blocks q of [i_q f_q o_q g_q] x 128
    # (reference gate order along 4H is [i, f, g, o])
    cols = []
    for q in range(NQ):
        for goff in (0, H, 3 * H, 2 * H):   # i, f, o, g
            s0 = goff + q * 128
            cols.extend(range(s0, s0 + 128))
    cols = np.array(cols)

    def tiles(w):
        return np.ascontiguousarray(
            w.reshape(KT, 128, MT, 128).transpose(1, 0, 2, 3)
            .reshape(128, KT * MT * 128)).astype(bfloat16)

    per_dir = {}
    for fwd, (Wih, bih, Whh, bhh) in (
            (True, (Wih_f, bih_f, Whh_f, bhh_f)),
            (False, (Wih_b, bih_b, Whh_b, bhh_b))):
        per_dir[fwd] = (
            tiles(Wih[:, cols]),
            tiles(Whh[:, cols]),
            np.ascontiguousarray(
                (bih + bhh)[cols].reshape(MT, 128).T).astype(np.float32),
        )

    in_maps = []
    for c in range(NCORES):
        fwd = c < 4
        qb = c & 3
        xs = x[qb * BL:(qb + 1) * BL]
        if not fwd:
            xs = xs[:, ::-1]
        xT = np.ascontiguousarray(
            xs.transpose(2, 1, 0).reshape(E, S * BL)).astype(bfloat16)
        wih_t, whh_t, bias_t = per_dir[fwd]
        in_maps.append({"xT": xT, "wih": wih_t, "whh": whh_t, "bias": bias_t})
    return in_maps


def _assemble(results):
    out = np.empty((B, S, 2 * H), np.float32)
    for c in range(NCORES):
        fwd = c < 4
        qb = c & 3
        arr = np.asarray(results[c]["stage"]).astype(np.float32)
        part = arr.transpose(3, 0, 2, 1).reshape(BL, S, NQ * 128)
        if not fwd:
            part = part[:, ::-1, :]
        base = 0 if fwd else H
        out[qb * BL:(qb + 1) * BL, :, base:base + H] = part
    return out


def kernel(x, Wih_f, bih_f, Whh_f, bhh_f, Wih_b, bih_b, Whh_b, bhh_b):
    global LAST_EXEC_NS
    if "nc" not in _cache:
        _cache["nc"] = _build_program()
    nc = _cache["nc"]
    in_maps = _host_inputs(np.asarray(x, np.float32),
                           np.asarray(Wih_f, np.float32),
                           np.asarray(bih_f, np.float32),
                           np.asarray(Whh_f, np.float32),
                           np.asarray(bhh_f, np.float32),
                           np.asarray(Wih_b, np.float32),
                           np.asarray(bih_b, np.float32),
                           np.asarray(Whh_b, np.float32),
                           np.asarray(bhh_b, np.float32))
    res = bass_utils.run_bass_kernel_spmd(nc, in_maps,
                                          core_ids=list(range(NCORES)),
                                          trace=TRACE)
    LAST_EXEC_NS = res.exec_time_ns
    return _assemble(res.results)

